# revision 39
# baseline (speedup 1.0000x reference)
"""Bass/Tile multi-head-attention kernel for Trainium2, SPMD over 8 NeuronCores.

Sparsity compaction: the reference multiplies the output rows by k_mas over the
*query* dim (out[q] = 0 whenever k_mas[q] = 0), and keys with k_mas[k] = 0 have
Kh = Vh = 0 so they contribute exactly att_mas[q,k]*exp(0) = att_mas[q,k] to the
softmax denominator and nothing to the numerator.  The host gathers
  - active queries  (k_mas[q] = 1) -> QCAP slots per core   (core = bs * 2 + half)
  - active keys     (k_mas[k] = 1) -> KCAP slots per batch
q_mas is folded into the gathered Q on the host (it zeroes Qh rows only).
The inactive keys' denominator contribution corr[q] = sum_inactive att_mas[q,k]
rides in as a *virtual key*: mask row KCAP-1 (always a padding slot) carries
corr[q] (fp16 keeps counts <= 2048 exact); its K/V columns are zero, so
S[k*,q] = 0, exp(0) = 1, and EP[k*,q] = corr[q] flows into the rowsum through
the regular ones-column AV accumulation while adding nothing to Y.  Pad-query
columns of that row carry 1.0 so their rowsum is 1 (keeps recip finite).

Device math per core (bs, half):
  QpT = WQ^T-contract (qm . Q_act)^T          [d, QCAP]
  KpT = WK^T-contract K_act^T                 [d, KCAP]
  Vp  = V_act^T proj, heads interleaved       [k, 8, 65] with ones col at 64
  per head h, q-block (512 | 64):
    S^T[k, q] = KpT_h^T-slice . QpT_h         (PE, contraction d=64; the two
                                               half outputs land at PSUM cols
                                               0/512 — one bank per matmul)
    E = exp(S^T / 8)                          (ACT, PSUM->SBUF, fp16)
    EP = E * Mact^T                           (DVE, fp16 2x)
    [Y^T; rowsum] += [Vp_h | 1]^T . EP        (PE, accumulated over kc)
    Y^T *= recip(rowsum)                      (DVE recip + Pool bcast + DVE mul)
  out^T = WO^T-contract . Y^T                 (PE)  -> DRAM [e, QCAP] fp32
Host scatters out^T columns back to the active query rows; inactive rows are 0.
"""

import numpy as np
import ml_dtypes

import concourse.bass as bass
import concourse.bacc as bacc
import concourse.mybir as mybir
import concourse.tile as tile
from concourse import bass_utils

BS, N, D, H, DK = 4, 2048, 512, 8, 64
NCORES = 8
NQH = N // 2          # raw queries per core before compaction
QCAP = 576            # active-query capacity per core      (mean 512, +4 sigma)
KCAP = 1152           # active-key capacity per bs          (mean 1024, +5.6 sigma;
                      # slot KCAP-1 is reserved for the corr virtual key)
KC = KCAP // 128      # 9 key chunks
QBLK = ((0, 512), (512, 64))                  # (offset, width) query blocks
KBLK = ((0, 512), (512, 512), (1024, 128))    # k-proj evacuation blocks
F32 = mybir.dt.float32
BF16 = mybir.dt.bfloat16
FP16 = mybir.dt.float16

CDT = BF16
NP_CDT = ml_dtypes.bfloat16

FLAGS = {
    "lookahead": 10,       # AV pipeline lookahead (groups)
    "pss_bufs": 3,
    "psy_bufs": 2,
    "proj_in_pss": True,
    "et_bufs": 8,
    "ep_bufs": 11,
    "warmup_mms": 8,     # dummy matmuls during the DMA ramp + ACT table preload
    "raw_dma": "tensor",  # qt/kt/vt granularity: "tensor" | "chunk" | "half"
    "exp_split": False,   # split the qb0 exp into two per-bank activations
    "wo_bg": False,        # emit WO(qb0) ec-groups as background tasks
    "v_bg": True,         # emit all V projections as background tasks
    "wo_pool": "psy",     # WO psum source: "pss" (proj pool) | "psy"
    "pevac": "dve",      # Q/K proj evacuation engine: "dve" | "pool"
    "vevac": "act",      # V proj evacuation engine: "act" | "pool" | "dve"
    "obf16": True,        # bf16 output tensor
    "kt_split": False,     # load kt in two column-range DMAs (cols 0:512 first)
    "mt_eng": "alt",      # mask DMA queue: "alt" (sync/gpsimd) | "gps" (all gpsimd)
    "spool_bufs": 4,      # scale-path scratch pool depth
}


def _chunked_ap(t_ap, rows, ncol, nch):
    """AP for loading a [rows*nch, ncol] DRAM tensor into one [rows, nch*ncol]
    SBUF tile, chunk-major along the free dim."""
    return bass.AP(tensor=t_ap.tensor, offset=t_ap.offset,
                   ap=[[ncol, rows], [rows * ncol, nch], [1, ncol]])


def _emit(nc, t):
    with tile.TileContext(nc) as tc:
        _emit_body(nc, tc, t)


def _emit_body(nc, tc, t, sfx=""):
    import contextlib
    ctx = contextlib.ExitStack()
    with ctx:
        persist = ctx.enter_context(tc.tile_pool(name="persist" + sfx, bufs=1))
        raw = ctx.enter_context(tc.tile_pool(name="raw" + sfx, bufs=1))

        # ---- weights: one [128, 4*D] tile per tensor, one DMA each ----
        w_tile = {}
        for wname in ("wqt", "wkt", "wvt", "wot"):
            w_tile[wname] = persist.tile([128, 4 * D], CDT, tag=wname, name=wname)

        def w_sb(wname, ct):
            return w_tile[wname][:, ct * D:(ct + 1) * D]

        def load_w(wname):
            nc.gpsimd.dma_start(out=w_tile[wname][:],
                                in_=_chunked_ap(t[wname].ap(), 128, D, 4))

        # ---- persistent result tensors ----
        qpt_sb = [persist.tile([128, QCAP], CDT, tag=f"qpt{i}", name=f"qpt{i}") for i in range(4)]
        kpt_sb = [persist.tile([128, KCAP], CDT, tag=f"kpt{i}", name=f"kpt{i}") for i in range(4)]
        v_sb = [persist.tile([128, H, DK + 1], FP16, tag=f"v{i}", name=f"v{i}") for i in range(KC)]
        yt_sb = [persist.tile([128, QCAP], CDT, tag=f"yt{i}", name=f"yt{i}") for i in range(4)]
        mt_ap = t["mt"].ap()

        # raw activations: one [128, 4*cols] tile per tensor
        raw_tile = {}
        for nm, cols in (("qt", QCAP), ("kt", KCAP), ("vt", KCAP)):
            raw_tile[nm] = raw.tile([128, 4 * cols], CDT, tag=nm, name=nm)

        def raw_sb(nm, ct):
            cols = QCAP if nm == "qt" else KCAP
            return raw_tile[nm][:, ct * cols:(ct + 1) * cols]

        def load_raw(nm):
            cols = QCAP if nm == "qt" else KCAP
            if nm == "kt" and FLAGS["kt_split"]:
                ap = t[nm].ap()
                for c0, cw in ((0, 512), (512, cols - 512)):
                    nc.sync.dma_start(
                        out=bass.AP(tensor=raw_tile[nm].tensor,
                                    offset=raw_tile[nm].offset + c0,
                                    ap=[list(raw_tile[nm].ap[0]), [cols, 4], [1, cw]]),
                        in_=bass.AP(tensor=ap.tensor, offset=ap.offset + c0,
                                    ap=[[cols, 128], [128 * cols, 4], [1, cw]]))
                return
            if FLAGS["raw_dma"] == "tensor":
                nc.sync.dma_start(out=raw_tile[nm][:],
                                  in_=_chunked_ap(t[nm].ap(), 128, cols, 4))
                return
            nhalf = 2 if FLAGS["raw_dma"] == "half" else 1
            cw = cols // nhalf
            for ct in range(4):
                for hlf in range(nhalf):
                    c0 = hlf * cw
                    nc.sync.dma_start(out=raw_tile[nm][:, ct * cols + c0:ct * cols + c0 + cw],
                                      in_=t[nm].ap()[ct * 128:(ct + 1) * 128, c0:c0 + cw])

        mpool = ctx.enter_context(tc.tile_pool(name="mts" + sfx, bufs=1))
        mts = {}

        def load_mt(kc):
            mtile = mpool.tile([128, QCAP], FP16, tag=f"mt{kc}", name=f"mt{kc}")
            eng = nc.gpsimd if (FLAGS["mt_eng"] == "gps" or kc % 2) else nc.sync
            eng.dma_start(out=mtile[:],
                          in_=bass.AP(tensor=mt_ap.tensor, offset=mt_ap.offset + kc * 128 * QCAP,
                                      ap=[[QCAP, 128], [1, QCAP]]))
            mts[kc] = mtile

        load_raw("kt")
        load_w("wkt")
        load_raw("qt")
        load_w("wqt")
        load_mt(0)
        load_raw("vt")
        load_mt(1)
        load_w("wvt")
        load_mt(2)
        load_w("wot")

        # bank budget: pss 2x2 + psy 3x1 + proj 1 = 8
        pool_s = ctx.enter_context(tc.tile_pool(name="pss" + sfx,
                                                bufs=int(FLAGS["pss_bufs"]), space="PSUM"))
        pp = None if FLAGS["proj_in_pss"] else \
            ctx.enter_context(tc.tile_pool(name="psproj" + sfx, bufs=1, space="PSUM"))

        def proj_ps():
            if pp is None:
                return pool_s.tile([128, 1024], F32, tag="pss", name="ps")[:, 0:512]
            return pp.tile([128, 512], F32, tag="ps", name="ps")

        def emit_proj_q_blk(dc, qb):
            q0, qw = QBLK[qb]
            ps = proj_ps()
            for ct in range(4):
                nc.tensor.matmul(ps[:, 0:qw], w_sb("wqt", ct)[:, dc * 128:(dc + 1) * 128],
                                 raw_sb("qt", ct)[:, q0:q0 + qw],
                                 start=(ct == 0), stop=(ct == 3))
            eng = nc.gpsimd if FLAGS["pevac"] == "pool" else nc.vector
            eng.tensor_copy(out=qpt_sb[dc][:, q0:q0 + qw], in_=ps[:, 0:qw])

        def emit_proj_k_blk(dc, kb):
            k0, kw = KBLK[kb]
            ps = proj_ps()
            for ct in range(4):
                nc.tensor.matmul(ps[:, 0:kw], w_sb("wkt", ct)[:, dc * 128:(dc + 1) * 128],
                                 raw_sb("kt", ct)[:, k0:k0 + kw],
                                 start=(ct == 0), stop=(ct == 3))
            eng = nc.gpsimd if FLAGS["pevac"] == "pool" else nc.vector
            eng.tensor_copy(out=kpt_sb[dc][:, k0:k0 + kw], in_=ps[:, 0:kw])

        def emit_proj_v(kc):
            ps = proj_ps()
            for ct in range(4):
                nc.tensor.matmul(ps[:], raw_sb("vt", ct)[:, kc * 128:(kc + 1) * 128],
                                 w_sb("wvt", ct), start=(ct == 0), stop=(ct == 3))
            veng = {"act": nc.scalar.copy, "pool": nc.gpsimd.tensor_copy,
                    "dve": nc.vector.tensor_copy}[FLAGS["vevac"]]
            veng(out=v_sb[kc][:, :, 0:DK],
                 in_=ps.rearrange("p (h e) -> p h e", h=H))
            nc.gpsimd.memset(v_sb[kc][:, :, DK:DK + 1], 1.0)

        # ---- attention pools ----
        pool_y = ctx.enter_context(tc.tile_pool(name="psy" + sfx, bufs=2, space="PSUM"))
        epool = ctx.enter_context(tc.tile_pool(name="eps" + sfx, bufs=1))
        spool = ctx.enter_context(tc.tile_pool(name="smalls" + sfx,
                                               bufs=int(FLAGS["spool_bufs"])))
        opool = ctx.enter_context(tc.tile_pool(name="osb" + sfx, bufs=3))

        psy = {}
        pending = []

        # PE/ACT warm-up during the input-DMA ramp
        nwarm = int(FLAGS["warmup_mms"])
        if nwarm:
            scratch = kpt_sb[0]
            nc.vector.memset(scratch[:, 0:512], 0.0)
            ps_w = proj_ps()
            for _ in range(nwarm):
                nc.tensor.matmul(ps_w[:], scratch[:, 0:128], scratch[:, 0:512],
                                 start=True, stop=True)
            nc.scalar.activation(out=yt_sb[0][:, 0:512], in_=ps_w[:],
                                 func=mybir.ActivationFunctionType.Exp, scale=0.125)

        def V(kc):
            return lambda: emit_proj_v(kc)

        def Kb(dc, kb):
            return lambda: emit_proj_k_blk(dc, kb)

        def Qb(dc, qb):
            return lambda: emit_proj_q_blk(dc, qb)

        def wo_ps():
            if FLAGS["wo_pool"] == "psy":
                return pool_y.tile([128, 512], F32, tag="psy", name="pso",
                                   bufs=int(FLAGS["psy_bufs"]))
            return proj_ps()

        def Wo(qb, ec):
            return lambda: _emit_wo_ec(nc, t, w_sb, yt_sb, wo_ps, opool, qb, ec)

        if FLAGS["v_bg"]:
            v_head = [V(kc) for kc in range(3)]
        else:
            v_head = []
            for kc in range(3):
                emit_proj_v(kc)
        kb_head = [Kb(0, 1), Kb(0, 2)] if FLAGS["kt_split"] else []
        bg_by_dc = {
            0: kb_head + [Qb(1, 0), Qb(1, 1), Kb(1, 0), Kb(1, 1)] + v_head
               + [V(3), V(4), V(5), Kb(1, 2), V(6), V(7), V(8)],
            1: [Qb(2, 0), Qb(2, 1), Kb(2, 0), Kb(2, 1), Kb(2, 2)],
            2: [Qb(3, 0), Qb(3, 1), Kb(3, 0), Kb(3, 1), Kb(3, 2)],
        }
        for kb in range(1 if FLAGS["kt_split"] else 3):
            emit_proj_k_blk(0, kb)
        for qb in range(2):
            emit_proj_q_blk(0, qb)

        def emit_av(qb, h0, kc, ep):
            q0, qw = QBLK[qb]
            for half, h in ((0, h0), (1, h0 + 1)):
                nc.tensor.matmul(psy[(h, qb)][0:DK + 1, 0:qw], v_sb[kc][:, h, :],
                                 ep[:, half * qw:(half + 1) * qw],
                                 start=(kc == 0), stop=(kc == KC - 1))
            if kc == KC - 1:
                for h in (h0, h0 + 1):
                    ps_y = psy[(h, qb)]
                    rec = spool.tile([1, 512], F32, tag="rec", name="rec")
                    nc.vector.reciprocal(rec[:, 0:qw], ps_y[DK:DK + 1, 0:qw])
                    sclb = spool.tile([DK, 512], F32, tag="sclb", name="sclb")
                    nc.gpsimd.partition_broadcast(sclb[:, 0:qw], rec[:, 0:qw])
                    po = (h % 2) * DK
                    nc.vector.tensor_mul(yt_sb[h // 2][po:po + DK, q0:q0 + qw],
                                         ps_y[0:DK, 0:qw], sclb[:, 0:qw])

        for dc in range(4):
            h0 = 2 * dc
            bg = bg_by_dc.get(dc, [])
            for qi, qb in enumerate((0, 1)):
                q0, qw = QBLK[qb]
                psy[(h0, qb)] = pool_y.tile([128, 512], F32, tag="psy",
                                            name="psy", bufs=int(FLAGS["psy_bufs"]))
                psy[(h0 + 1, qb)] = pool_y.tile([128, 512], F32, tag="psy",
                                                name="psy", bufs=int(FLAGS["psy_bufs"]))
                for kc in range(KC):
                    if kc not in mts:
                        load_mt(kc)
                    # half outputs live at bank-aligned cols 0 / 512 (one bank
                    # per matmul output; sub-bank offsets abort on HW)
                    ps_s = pool_s.tile([128, 1024], F32, tag="pss", name="ps_s")
                    for half in range(2):
                        po = half * DK
                        nc.tensor.matmul(ps_s[:, half * 512:half * 512 + qw],
                                         kpt_sb[dc][po:po + DK, kc * 128:(kc + 1) * 128],
                                         qpt_sb[dc][po:po + DK, q0:q0 + qw],
                                         start=True, stop=True)
                    if bg:
                        bg.pop(0)()
                    et = epool.tile([128, 1024], FP16, tag="et", name="et",
                                    bufs=int(FLAGS["et_bufs"]))
                    if qw == 512 and FLAGS["exp_split"]:
                        for half in range(2):
                            nc.scalar.activation(
                                out=et[:, half * 512:(half + 1) * 512],
                                in_=ps_s[:, half * 512:(half + 1) * 512],
                                func=mybir.ActivationFunctionType.Exp, scale=0.125)
                    else:
                        exp_in = ps_s[:, 0:2 * qw] if qw == 512 else \
                            bass.AP(tensor=ps_s.tensor, offset=ps_s.offset,
                                    ap=[list(ps_s.ap[0]), [512, 2], [1, qw]])
                        nc.scalar.activation(out=et[:, 0:2 * qw], in_=exp_in,
                                             func=mybir.ActivationFunctionType.Exp,
                                             scale=0.125)
                    ep = epool.tile([128, 1024], FP16, tag="ep", name="ep",
                                    bufs=int(FLAGS["ep_bufs"]))
                    mtap = mts[kc]
                    mb = bass.AP(tensor=mtap.tensor, offset=mtap.offset + q0,
                                 ap=[list(mtap.ap[0]), [0, 2], [1, qw]])
                    nc.vector.tensor_mul(
                        ep[:, 0:2 * qw].rearrange("p (a b) -> p a b", a=2),
                        et[:, 0:2 * qw].rearrange("p (a b) -> p a b", a=2), mb)
                    pending.append((qb, h0, kc, ep))
                    if len(pending) > int(FLAGS["lookahead"]):
                        emit_av(*pending.pop(0))
                if dc == 3 and qi == 0:
                    while pending:
                        emit_av(*pending.pop(0))
                    if FLAGS["wo_bg"]:
                        bg.extend(Wo(qb, ec) for ec in range(4))
                    else:
                        odt = BF16 if FLAGS["obf16"] else F32
                        ot_w0 = opool.tile([128, 4 * 512], odt, tag="otw0", name="otw0")
                        for ec in range(4):
                            _emit_wo_ec(nc, t, w_sb, yt_sb, wo_ps, opool, qb, ec,
                                        ot_w=ot_w0)
        while pending:
            emit_av(*pending.pop(0))
        # final (64-wide) WO: alternate free psum pools, evacuate the four ec
        # chunks into one wide tile, ship with a single DMA
        q0, qw = QBLK[1]
        odt = BF16 if FLAGS["obf16"] else F32
        ot_w = opool.tile([128, 4 * qw], odt, tag="otw", name="otw")
        for ec in range(4):
            ps = wo_ps() if ec % 2 else proj_ps()
            for dt_ in range(4):
                nc.tensor.matmul(ps[:, 0:qw], w_sb("wot", dt_)[:, ec * 128:(ec + 1) * 128],
                                 yt_sb[dt_][:, q0:q0 + qw],
                                 start=(dt_ == 0), stop=(dt_ == 3))
            nc.scalar.copy(out=ot_w[:, ec * qw:(ec + 1) * qw], in_=ps[:, 0:qw])
        oap = t["out_t"].ap()
        nc.sync.dma_start(
            out=bass.AP(tensor=oap.tensor, offset=oap.offset + q0,
                        ap=[[QCAP, 128], [128 * QCAP, 4], [1, qw]]),
            in_=ot_w[:])


def _emit_wo_ec(nc, t, w_sb, yt_sb, wo_ps, opool, qb, ec, ot_w=None):
    q0, qw = QBLK[qb]
    odt = BF16 if FLAGS["obf16"] else F32
    ps = wo_ps()
    for dt_ in range(4):
        nc.tensor.matmul(ps[:, 0:qw], w_sb("wot", dt_)[:, ec * 128:(ec + 1) * 128],
                         yt_sb[dt_][:, q0:q0 + qw],
                         start=(dt_ == 0), stop=(dt_ == 3))
    if ot_w is not None:
        nc.scalar.copy(out=ot_w[:, ec * qw:(ec + 1) * qw], in_=ps[:, 0:qw])
        if ec == 3:
            oap = t["out_t"].ap()
            nc.sync.dma_start(
                out=bass.AP(tensor=oap.tensor, offset=oap.offset + q0,
                            ap=[[QCAP, 128], [128 * QCAP, 4], [1, qw]]),
                in_=ot_w[:])
        return
    ot = opool.tile([128, 512], odt, tag="ot", name="ot")
    nc.scalar.copy(out=ot[:, 0:qw], in_=ps[:, 0:qw])
    nc.sync.dma_start(out=t["out_t"].ap()[ec * 128:(ec + 1) * 128, q0:q0 + qw],
                      in_=ot[:, 0:qw])


_NC_CACHE = {}


def build():
    if "nc" in _NC_CACHE:
        return _NC_CACHE["nc"], _NC_CACHE["t"]
    nc = bacc.Bacc(None, target_bir_lowering=False, debug=False)
    t = {
        "qt": nc.dram_tensor("qt", [D, QCAP], CDT, kind="ExternalInput"),
        "kt": nc.dram_tensor("kt", [D, KCAP], CDT, kind="ExternalInput"),
        "vt": nc.dram_tensor("vt", [D, KCAP], CDT, kind="ExternalInput"),
        "mt": nc.dram_tensor("mt", [KCAP, QCAP], FP16, kind="ExternalInput"),
        "wqt": nc.dram_tensor("wqt", [D, D], CDT, kind="ExternalInput"),
        "wkt": nc.dram_tensor("wkt", [D, D], CDT, kind="ExternalInput"),
        "wvt": nc.dram_tensor("wvt", [D, D], CDT, kind="ExternalInput"),
        "wot": nc.dram_tensor("wot", [D, D], CDT, kind="ExternalInput"),
        "out_t": nc.dram_tensor("out_t", [D, QCAP],
                                BF16 if FLAGS["obf16"] else F32, kind="ExternalOutput"),
    }
    _emit(nc, t)
    nc.compile()
    _NC_CACHE["nc"] = nc
    _NC_CACHE["t"] = t
    return nc, t


def make_in_maps(Q, K, V, q_mas, k_mas, att_mas, WQ, WK, WV, WO):
    """Returns (in_maps, plans); plans[c] = (bs, act_q_indices) for unsharding."""
    Q, K, V = (np.asarray(x, np.float32) for x in (Q, K, V))
    q_mas = np.asarray(q_mas, np.float32).reshape(BS, N)
    k_mas = np.asarray(k_mas, np.float32).reshape(BS, N)
    att_mas = np.asarray(att_mas, np.float32)
    wqt = np.ascontiguousarray(np.asarray(WQ, np.float32).T).astype(NP_CDT)
    wkt = np.ascontiguousarray(np.asarray(WK, np.float32).T).astype(NP_CDT)
    wvt = np.ascontiguousarray(np.asarray(WV, np.float32).T).astype(NP_CDT)
    wot = np.ascontiguousarray(np.asarray(WO, np.float32).T).astype(NP_CDT)
    in_maps, plans = [], []
    for bs in range(BS):
        km = k_mas[bs]
        act_k = np.flatnonzero(km > 0.5)
        ina_k = np.flatnonzero(km <= 0.5)
        nk = len(act_k)
        assert nk <= KCAP - 1, nk  # slot KCAP-1 reserved for the corr virtual key
        ktc = np.zeros((D, KCAP), NP_CDT)
        ktc[:, :nk] = K[bs].T[:, act_k].astype(NP_CDT)
        vtc = np.zeros((D, KCAP), NP_CDT)
        vtc[:, :nk] = V[bs].T[:, act_k].astype(NP_CDT)
        am = att_mas[bs]  # [q, k]
        for half in range(2):
            q0 = half * NQH
            act_q = q0 + np.flatnonzero(km[q0:q0 + NQH] > 0.5)
            nq = len(act_q)
            assert nq <= QCAP, nq
            qtc = np.zeros((D, QCAP), NP_CDT)
            qtc[:, :nq] = (Q[bs].T[:, act_q] * q_mas[bs, act_q][None, :]).astype(NP_CDT)
            mt = np.zeros((KCAP, QCAP), np.float16)
            mt[:nk, :nq] = am[np.ix_(act_q, act_k)].T.astype(np.float16)
            # corr virtual key: S row is 0 => exp = 1 => EP = corr -> rowsum
            mt[KCAP - 1, :nq] = am[act_q][:, ina_k].sum(axis=1).astype(np.float16)
            mt[KCAP - 1, nq:] = 1.0  # pad-query rowsum guard
            in_maps.append({
                "qt": qtc, "kt": ktc, "vt": vtc, "mt": mt,
                "wqt": wqt, "wkt": wkt, "wvt": wvt, "wot": wot,
            })
            plans.append((bs, act_q))
    return in_maps, plans


def unshard(results, plans):
    out = np.zeros((BS, N, D), np.float32)
    for c in range(NCORES):
        bs, act_q = plans[c]
        nq = len(act_q)
        out[bs, act_q, :] = np.asarray(results[c]["out_t"], np.float32)[:, :nq].T
    return out


def kernel(Q, K, V, q_mas, k_mas, att_mas, WQ, WK, WV, WO):
    nc, _ = build()
    in_maps, plans = make_in_maps(Q, K, V, q_mas, k_mas, att_mas, WQ, WK, WV, WO)
    res = bass_utils.run_bass_kernel_spmd(nc, in_maps, core_ids=list(range(NCORES)))
    return unshard(res.results, plans)
